# revision 6
# baseline (speedup 1.0000x reference)
"""Fused single-head attention (QKV projection + softmax(QK^T)V) on 8 trn2 cores.

Problem (hardcoded): x [4, 4096, 768] f32, W_qkv [768, 2304] f32, b_qkv [2304] f32.
  qkv = x @ W_qkv + b_qkv ; q,k,v = split(qkv, 3)
  out = softmax(q k^T / sqrt(768)) v          -> [4, 4096, 768] f32

Sharding: batch (4) x query-halves (2) -> 8 cores. Each core gets one batch's
x (pre-transposed on host to xT [768, 4096] fp16) and computes k/v for all
4096 keys plus q/attention for its 2048 queries. Output per core is
outT [768, 2048] f32 (transposed back on host).

On-chip layout ("transposed flash attention"):
  - qkv computed in head-major layout qT/kT [H, n] via lhsT=W, rhs=xT; v in
    [n, H] via lhsT=xT, rhs=W (no on-chip transposes anywhere).
  - scores computed transposed: sT[j, i] = (kT j-tile).T @ qT -> PSUM,
    exp via ScalarE (scale folded in), p stored fp16.
  - softmax denominator: ones[128,128] @ p accumulates denom broadcast
    across all partitions -> PSUM.
  - outT[h, i] += (v j-tile).T @ p accumulated over j in PSUM, then
    normalized by DVE tensor_tensor multiply with reciprocal(denom).
No max-subtraction is needed: scores are O(1) here, exp is safe in fp32/fp16.
"""

import math
from contextlib import ExitStack
from functools import lru_cache

import numpy as np

import concourse.bacc as bacc
import concourse.bass as bass
import concourse.tile as tile
from concourse import mybir
from concourse.bass_utils import run_bass_kernel_spmd

B, N, C = 4, 4096, 768
H = 768          # head dim (== C)
H3 = 3 * H
NCORES = 8
NQ = N // 2      # queries per core
DT = mybir.dt.float16
F32 = mybir.dt.float32
SCALE = 1.0 / math.sqrt(H)

CT = C // 128    # 6 contraction tiles (c)
HT = H // 128    # 6 head tiles (h)
JT = N // 128    # 32 key tiles (j)
RB = 8           # r-blocks of 512 over the 4096 rows
RBS = N // RB    # 512
IB = 4           # i-blocks of 512 over this core's 2048 queries
IBS = NQ // IB   # 512


def build_program():
    nc = bacc.Bacc(
        "TRN2",
        target_bir_lowering=False,
        debug=False,
        enable_asserts=False,
        num_devices=NCORES,
    )
    xT_d = nc.dram_tensor("xT", [C, N], DT, kind="ExternalInput").ap()
    w_d = nc.dram_tensor("w", [C, H3], DT, kind="ExternalInput").ap()
    bqk_d = nc.dram_tensor("bqk", [128, 2 * HT], F32, kind="ExternalInput").ap()
    bv_d = nc.dram_tensor("bv", [128, H], F32, kind="ExternalInput").ap()
    outT_d = nc.dram_tensor("outT", [H, NQ], F32, kind="ExternalOutput").ap()

    with tile.TileContext(nc) as tc:
        with ExitStack() as ctx:
            persist = ctx.enter_context(tc.tile_pool(name="persist", bufs=1))

            # Persistent SBUF tensors (bytes/partition): kT 48K, qT 24K, v 48K
            kT = [persist.tile([128, N], DT, tag=f"kT{t}", name=f"kT{t}") for t in range(HT)]
            qT = [persist.tile([128, NQ], DT, tag=f"qT{t}", name=f"qT{t}") for t in range(HT)]
            vv = [persist.tile([128, H], DT, tag=f"v{t}", name=f"v{t}") for t in range(JT)]
            ones = persist.tile([128, 128], DT, tag="ones")
            nc.vector.memset(ones, 1.0)
            bqk = persist.tile([128, 2 * HT], F32, tag="bqk")
            nc.sync.dma_start(out=bqk, in_=bqk_d)
            bvb = persist.tile([128, H], F32, tag="bvb")
            nc.sync.dma_start(out=bvb, in_=bv_d)

            # ---- Phase 1: QKV projection ----
            with tc.tile_pool(name="wpool", bufs=1) as wpool, \
                 tc.tile_pool(name="xpool", bufs=2 * CT) as xpool, \
                 tc.tile_pool(name="pj", bufs=4, space="PSUM") as pj, \
                 tc.tile_pool(name="pv", bufs=2, space="PSUM") as pv:

                ws = [wpool.tile([128, H3], DT, tag=f"w{t}", name=f"w{t}") for t in range(CT)]
                for ct in range(CT):
                    nc.sync.dma_start(out=ws[ct], in_=w_d[ct * 128:(ct + 1) * 128, :])

                for rb in range(RB):
                    r0 = rb * RBS
                    xt = []
                    for ct in range(CT):
                        t = xpool.tile([128, RBS], DT, tag="xt", name=f"xt{rb}_{ct}")
                        nc.sync.dma_start(
                            out=t, in_=xT_d[ct * 128:(ct + 1) * 128, r0:r0 + RBS])
                        xt.append(t)

                    # kT (and qT if this r-block is in our query half)
                    projs = [(H, kT, 0, r0)]
                    qh_r0 = r0 - (NQ if rb >= RB // 2 else 0)
                    # query half selected per-core at runtime? No -- program is
                    # identical across cores; core's query half is baked via
                    # host-supplied xT/q mapping. We compute q for rows
                    # [qsel*NQ, qsel*NQ+NQ) -- but the program is shared, so q
                    # is taken from the FIRST half for even cores... Instead:
                    # host maps this core's query rows into xT columns
                    # [0, NQ) by swapping halves (see kernel()). So q always
                    # comes from columns [0, NQ).
                    if rb < RB // 2:
                        projs.append((0, qT, 0, r0))
                    for (wofs, dst, dofs, c0) in projs:
                        for ht in range(HT):
                            ps = pj.tile([128, RBS], F32, tag="pj")
                            for ct in range(CT):
                                nc.tensor.matmul(
                                    ps,
                                    ws[ct][:, wofs + ht * 128: wofs + (ht + 1) * 128],
                                    xt[ct],
                                    start=(ct == 0), stop=(ct == CT - 1),
                                )
                            bcol = (0 if wofs == 0 else HT) + ht
                            nc.scalar.activation(
                                out=dst[ht][:, dofs + c0: dofs + c0 + RBS],
                                in_=ps,
                                func=mybir.ActivationFunctionType.Identity,
                                bias=bqk[:, bcol:bcol + 1],
                            )

                    # v for the 4 j-tiles of this r-block
                    for j in range(RBS // 128):
                        jt = rb * (RBS // 128) + j
                        ps = pv.tile([128, H], F32, tag="pv")
                        for ct in range(CT):
                            xs = xt[ct][:, j * 128:(j + 1) * 128]
                            nc.tensor.matmul(
                                ps[:, 0:512], xs, ws[ct][:, 2 * H: 2 * H + 512],
                                start=(ct == 0), stop=(ct == CT - 1))
                            nc.tensor.matmul(
                                ps[:, 512:H], xs, ws[ct][:, 2 * H + 512: 3 * H],
                                start=(ct == 0), stop=(ct == CT - 1))
                        nc.vector.tensor_add(vv[jt], ps, bvb)

            # ---- Phase 2: attention ----
            with tc.tile_pool(name="ppool", bufs=1) as ppool, \
                 tc.tile_pool(name="opool", bufs=4) as opool:
                p_t = [ppool.tile([128, IBS], DT, tag=f"p{t}", name=f"p{t}") for t in range(JT)]

                for ib in range(IB):
                    i0 = ib * IBS
                    # pass 1: scores^T -> exp -> p, denominator accumulation
                    with tc.tile_pool(name="ps_s", bufs=2, space="PSUM") as ps_s, \
                         tc.tile_pool(name="ps_d", bufs=1, space="PSUM") as ps_d:
                        dps = ps_d.tile([128, IBS], F32, tag="d")
                        for jt in range(JT):
                            sps = ps_s.tile([128, IBS], F32, tag="s")
                            for ht in range(HT):
                                nc.tensor.matmul(
                                    sps,
                                    kT[ht][:, jt * 128:(jt + 1) * 128],
                                    qT[ht][:, i0:i0 + IBS],
                                    start=(ht == 0), stop=(ht == HT - 1),
                                )
                            nc.scalar.activation(
                                out=p_t[jt], in_=sps,
                                func=mybir.ActivationFunctionType.Exp,
                                scale=SCALE,
                            )
                            nc.tensor.matmul(
                                dps, ones, p_t[jt],
                                start=(jt == 0), stop=(jt == JT - 1),
                            )
                        rcp = opool.tile([128, IBS], F32, tag="rcp")
                        nc.vector.reciprocal(out=rcp, in_=dps)

                    # pass 2: outT accumulation over j, then normalize
                    with tc.tile_pool(name="ps_o", bufs=6, space="PSUM") as ps_o:
                        ops = [ps_o.tile([128, IBS], F32, tag="o", name=f"o{ib}_{_}") for _ in range(HT)]
                        for jt in range(JT):
                            for ht in range(HT):
                                nc.tensor.matmul(
                                    ops[ht],
                                    vv[jt][:, ht * 128:(ht + 1) * 128],
                                    p_t[jt],
                                    start=(jt == 0), stop=(jt == JT - 1),
                                )
                        for ht in range(HT):
                            ot = opool.tile([128, IBS], F32, tag="ot")
                            nc.vector.tensor_tensor(
                                out=ot, in0=ops[ht], in1=rcp,
                                op=mybir.AluOpType.mult)
                            nc.sync.dma_start(
                                out=outT_d[ht * 128:(ht + 1) * 128, i0:i0 + IBS],
                                in_=ot)
    nc.compile()
    return nc


@lru_cache(maxsize=1)
def _cached_program():
    return build_program()


def kernel(x, W_qkv, b_qkv):
    nc = _cached_program()

    w16 = W_qkv.astype(np.float16)
    bq = b_qkv[0:H].astype(np.float32).reshape(HT, 128).T    # [128, HT]
    bk = b_qkv[H:2 * H].astype(np.float32).reshape(HT, 128).T
    bqk = np.ascontiguousarray(np.concatenate([bq, bk], axis=1))  # [128, 2*HT]
    bv = np.ascontiguousarray(
        np.broadcast_to(b_qkv[2 * H:3 * H].astype(np.float32), (128, H)))

    in_maps = []
    for core in range(NCORES):
        b, qh = core // 2, core % 2
        xb = x[b]  # [N, C] f32
        if qh == 1:
            # Swap halves so this core's query rows occupy columns [0, NQ)
            # of xT; key/value rows cover all of x (order along j does not
            # matter for attention as long as k and v agree).
            xb = np.concatenate([xb[NQ:], xb[:NQ]], axis=0)
        xT = np.ascontiguousarray(xb.T).astype(np.float16)
        in_maps.append({"xT": xT, "w": w16, "bqk": bqk, "bv": bv})

    res = run_bass_kernel_spmd(nc, in_maps, core_ids=list(range(NCORES)))

    out = np.empty((B, N, C), dtype=np.float32)
    for core in range(NCORES):
        b, qh = core // 2, core % 2
        outT = res.results[core]["outT"]  # [H, NQ] f32
        out[b, qh * NQ:(qh + 1) * NQ, :] = outT.T
    return out


# revision 8
# speedup vs baseline: 1.0102x; 1.0102x over previous
"""Fused single-head attention (QKV projection + softmax(QK^T)V) on 8 trn2 cores.

Problem (hardcoded): x [4, 4096, 768] f32, W_qkv [768, 2304] f32, b_qkv [2304] f32.
  qkv = x @ W_qkv + b_qkv ; q,k,v = split(qkv, 3)
  out = softmax(q k^T / sqrt(768)) v          -> [4, 4096, 768] f32

Sharding: batch (4) x query-halves (2) -> 8 cores. Each core gets one batch's
x (pre-transposed on host to xT [768, 4096] fp16) and computes k/v for all
4096 keys plus q/attention for its 2048 queries. Output per core is
outT [768, 2048] f32 (transposed back on host).

On-chip layout ("transposed flash attention"):
  - qkv computed in head-major layout qT/kT [H, n] via lhsT=W, rhs=xT; v in
    [n, H] via lhsT=xT, rhs=W (no on-chip transposes anywhere).
  - scores computed transposed: sT[j, i] = (kT j-tile).T @ qT -> PSUM,
    exp via ScalarE (scale folded in), p stored fp16.
  - softmax denominator: ones[128,128] @ p accumulates denom broadcast
    across all partitions -> PSUM.
  - outT[h, i] += (v j-tile).T @ p accumulated over j in PSUM, then
    normalized by DVE tensor_tensor multiply with reciprocal(denom).
No max-subtraction is needed: scores are O(1) here, exp is safe in fp32/fp16.
"""

import math
from contextlib import ExitStack
from functools import lru_cache

import numpy as np

import concourse.bacc as bacc
import concourse.bass as bass
import concourse.tile as tile
from concourse import mybir
from concourse.bass_utils import run_bass_kernel_spmd

B, N, C = 4, 4096, 768
H = 768          # head dim (== C)
H3 = 3 * H
NCORES = 8
NQ = N // 2      # queries per core
DT = mybir.dt.float16
F32 = mybir.dt.float32
SCALE = 1.0 / math.sqrt(H)

CT = C // 128    # 6 contraction tiles (c)
HT = H // 128    # 6 head tiles (h)
JT = N // 128    # 32 key tiles (j)
RB = 8           # r-blocks of 512 over the 4096 rows
RBS = N // RB    # 512
IB = 4           # i-blocks of 512 over this core's 2048 queries
IBS = NQ // IB   # 512


def build_program():
    nc = bacc.Bacc(
        "TRN2",
        target_bir_lowering=False,
        debug=False,
        enable_asserts=False,
        num_devices=NCORES,
    )
    xT_d = nc.dram_tensor("xT", [C, N], DT, kind="ExternalInput").ap()
    w_d = nc.dram_tensor("w", [C, H3], DT, kind="ExternalInput").ap()
    bqk_d = nc.dram_tensor("bqk", [128, 2 * HT], F32, kind="ExternalInput").ap()
    bv_d = nc.dram_tensor("bv", [128, H], F32, kind="ExternalInput").ap()
    outT_d = nc.dram_tensor("outT", [H, NQ], F32, kind="ExternalOutput").ap()

    with tile.TileContext(nc) as tc:
        with ExitStack() as ctx:
            persist = ctx.enter_context(tc.tile_pool(name="persist", bufs=1))

            # Persistent SBUF tensors (bytes/partition): kT 48K, qT 24K, v 48K
            kT = [persist.tile([128, N], DT, tag=f"kT{t}", name=f"kT{t}") for t in range(HT)]
            qT = [persist.tile([128, NQ], DT, tag=f"qT{t}", name=f"qT{t}") for t in range(HT)]
            vv = [persist.tile([128, H], DT, tag=f"v{t}", name=f"v{t}") for t in range(JT)]
            ones = persist.tile([128, 128], DT, tag="ones")
            nc.vector.memset(ones, 1.0)
            bqk = persist.tile([128, 2 * HT], F32, tag="bqk")
            nc.sync.dma_start(out=bqk, in_=bqk_d)
            bvb = persist.tile([128, H], F32, tag="bvb")
            nc.sync.dma_start(out=bvb, in_=bv_d)

            # ---- Phase 1: QKV projection ----
            with tc.tile_pool(name="wpool", bufs=1) as wpool, \
                 tc.tile_pool(name="xpool", bufs=2 * CT) as xpool, \
                 tc.tile_pool(name="pj", bufs=4, space="PSUM") as pj, \
                 tc.tile_pool(name="pv", bufs=2, space="PSUM") as pv:

                ws = [wpool.tile([128, H3], DT, tag=f"w{t}", name=f"w{t}") for t in range(CT)]

                def load_xt(rb):
                    r0 = rb * RBS
                    tiles = []
                    for ct in range(CT):
                        t = xpool.tile([128, RBS], DT, tag="xt", name=f"xt{rb}_{ct}")
                        nc.sync.dma_start(
                            out=t, in_=xT_d[ct * 128:(ct + 1) * 128, r0:r0 + RBS])
                        tiles.append(t)
                    return tiles

                # first r-block's x and the k-projection columns of W land
                # first so the PE can start as early as possible
                xt0 = load_xt(0)
                for ct in range(CT):
                    nc.sync.dma_start(out=ws[ct][:, H:2 * H],
                                      in_=w_d[ct * 128:(ct + 1) * 128, H:2 * H])
                for ct in range(CT):
                    nc.sync.dma_start(out=ws[ct][:, 0:H],
                                      in_=w_d[ct * 128:(ct + 1) * 128, 0:H])
                    nc.sync.dma_start(out=ws[ct][:, 2 * H:H3],
                                      in_=w_d[ct * 128:(ct + 1) * 128, 2 * H:H3])

                for rb in range(RB):
                    r0 = rb * RBS
                    xt = xt0 if rb == 0 else load_xt(rb)

                    # kT (and qT if this r-block is in our query half)
                    projs = [(H, kT, 0, r0)]
                    qh_r0 = r0 - (NQ if rb >= RB // 2 else 0)
                    # query half selected per-core at runtime? No -- program is
                    # identical across cores; core's query half is baked via
                    # host-supplied xT/q mapping. We compute q for rows
                    # [qsel*NQ, qsel*NQ+NQ) -- but the program is shared, so q
                    # is taken from the FIRST half for even cores... Instead:
                    # host maps this core's query rows into xT columns
                    # [0, NQ) by swapping halves (see kernel()). So q always
                    # comes from columns [0, NQ).
                    if rb < RB // 2:
                        projs.append((0, qT, 0, r0))
                    for (wofs, dst, dofs, c0) in projs:
                        for ht in range(HT):
                            ps = pj.tile([128, RBS], F32, tag="pj")
                            for ct in range(CT):
                                nc.tensor.matmul(
                                    ps,
                                    ws[ct][:, wofs + ht * 128: wofs + (ht + 1) * 128],
                                    xt[ct],
                                    start=(ct == 0), stop=(ct == CT - 1),
                                )
                            bcol = (0 if wofs == 0 else HT) + ht
                            nc.scalar.activation(
                                out=dst[ht][:, dofs + c0: dofs + c0 + RBS],
                                in_=ps,
                                func=mybir.ActivationFunctionType.Identity,
                                bias=bqk[:, bcol:bcol + 1],
                            )

                    # v for the 4 j-tiles of this r-block
                    for j in range(RBS // 128):
                        jt = rb * (RBS // 128) + j
                        ps = pv.tile([128, H], F32, tag="pv")
                        for ct in range(CT):
                            xs = xt[ct][:, j * 128:(j + 1) * 128]
                            nc.tensor.matmul(
                                ps[:, 0:512], xs, ws[ct][:, 2 * H: 2 * H + 512],
                                start=(ct == 0), stop=(ct == CT - 1))
                            nc.tensor.matmul(
                                ps[:, 512:H], xs, ws[ct][:, 2 * H + 512: 3 * H],
                                start=(ct == 0), stop=(ct == CT - 1))
                        nc.vector.tensor_add(vv[jt], ps, bvb)

            # ---- Phase 2: attention ----
            # PSUM budget (8 banks): s 1 + d 1 + o 6.  Output h-tiles are
            # processed in two groups of 3 so that group g's PSUM
            # accumulators can be evacuated (DVE) while group g+1's matmuls
            # run -- the PE never waits at i-block boundaries.
            HG = HT // 2  # 3 h-tiles per group
            with tc.tile_pool(name="ppool", bufs=1) as ppool, \
                 tc.tile_pool(name="opool", bufs=4) as opool, \
                 tc.tile_pool(name="ps_s", bufs=1, space="PSUM") as ps_s, \
                 tc.tile_pool(name="ps_d", bufs=1, space="PSUM") as ps_d, \
                 tc.tile_pool(name="ps_o", bufs=6, space="PSUM") as ps_o:
                p_t = [ppool.tile([128, IBS], DT, tag=f"p{t}", name=f"p{t}") for t in range(JT)]

                def evac(ops_g, hts, rcp, i0):
                    for ot_ps, ht in zip(ops_g, hts):
                        ot = opool.tile([128, IBS], F32, tag="ot", name=f"ot{i0}_{ht}")
                        nc.vector.tensor_tensor(
                            out=ot, in0=ot_ps, in1=rcp,
                            op=mybir.AluOpType.mult)
                        nc.sync.dma_start(
                            out=outT_d[ht * 128:(ht + 1) * 128, i0:i0 + IBS],
                            in_=ot)

                for ib in range(IB):
                    i0 = ib * IBS
                    # group 0: scores -> exp -> p, denominator, PV for h 0-2
                    dps = ps_d.tile([128, IBS], F32, tag="d")
                    og0 = [ps_o.tile([128, IBS], F32, tag="o", name=f"og0_{ib}_{g}")
                           for g in range(HG)]
                    for jt in range(JT):
                        sps = ps_s.tile([128, IBS], F32, tag="s")
                        for ht in range(HT):
                            nc.tensor.matmul(
                                sps,
                                kT[ht][:, jt * 128:(jt + 1) * 128],
                                qT[ht][:, i0:i0 + IBS],
                                start=(ht == 0), stop=(ht == HT - 1),
                            )
                        nc.scalar.activation(
                            out=p_t[jt], in_=sps,
                            func=mybir.ActivationFunctionType.Exp,
                            scale=SCALE,
                        )
                        nc.tensor.matmul(
                            dps, ones, p_t[jt],
                            start=(jt == 0), stop=(jt == JT - 1),
                        )
                        for g in range(HG):
                            nc.tensor.matmul(
                                og0[g],
                                vv[jt][:, g * 128:(g + 1) * 128],
                                p_t[jt],
                                start=(jt == 0), stop=(jt == JT - 1),
                            )
                    rcp = opool.tile([128, IBS], F32, tag="rcp", name=f"rcp{ib}")
                    nc.vector.reciprocal(out=rcp, in_=dps)
                    evac(og0, range(HG), rcp, i0)

                    # group 1: PV for h 3-5 (evac of group 0 overlaps this)
                    og1 = [ps_o.tile([128, IBS], F32, tag="o", name=f"og1_{ib}_{g}")
                           for g in range(HG)]
                    for jt in range(JT):
                        for g in range(HG):
                            nc.tensor.matmul(
                                og1[g],
                                vv[jt][:, (HG + g) * 128:(HG + g + 1) * 128],
                                p_t[jt],
                                start=(jt == 0), stop=(jt == JT - 1),
                            )
                    evac(og1, range(HG, HT), rcp, i0)
    nc.compile()
    return nc


@lru_cache(maxsize=1)
def _cached_program():
    return build_program()


def kernel(x, W_qkv, b_qkv):
    nc = _cached_program()

    w16 = W_qkv.astype(np.float16)
    bq = b_qkv[0:H].astype(np.float32).reshape(HT, 128).T    # [128, HT]
    bk = b_qkv[H:2 * H].astype(np.float32).reshape(HT, 128).T
    bqk = np.ascontiguousarray(np.concatenate([bq, bk], axis=1))  # [128, 2*HT]
    bv = np.ascontiguousarray(
        np.broadcast_to(b_qkv[2 * H:3 * H].astype(np.float32), (128, H)))

    in_maps = []
    for core in range(NCORES):
        b, qh = core // 2, core % 2
        xb = x[b]  # [N, C] f32
        if qh == 1:
            # Swap halves so this core's query rows occupy columns [0, NQ)
            # of xT; key/value rows cover all of x (order along j does not
            # matter for attention as long as k and v agree).
            xb = np.concatenate([xb[NQ:], xb[:NQ]], axis=0)
        xT = np.ascontiguousarray(xb.T).astype(np.float16)
        in_maps.append({"xT": xT, "w": w16, "bqk": bqk, "bv": bv})

    res = run_bass_kernel_spmd(nc, in_maps, core_ids=list(range(NCORES)))

    out = np.empty((B, N, C), dtype=np.float32)
    for core in range(NCORES):
        b, qh = core // 2, core % 2
        outT = res.results[core]["outT"]  # [H, NQ] f32
        out[b, qh * NQ:(qh + 1) * NQ, :] = outT.T
    return out


# revision 10
# speedup vs baseline: 1.2037x; 1.1916x over previous
"""Fused single-head attention (QKV projection + softmax(QK^T)V) on 8 trn2 cores.

Problem (hardcoded): x [4, 4096, 768] f32, W_qkv [768, 2304] f32, b_qkv [2304] f32.
  qkv = x @ W_qkv + b_qkv ; q,k,v = split(qkv, 3)
  out = softmax(q k^T / sqrt(768)) v          -> [4, 4096, 768] f32

Sharding: batch (4) x key-halves (2) -> 8 cores. Each core gets one batch's
x (pre-transposed on host to xT [768, 4096] fp16, with the key half it owns
rotated into columns [0, 2048)), projects q for all 4096 queries but k/v only
for its 2048 keys, and computes PARTIAL attention sums over those keys:
  outT_partial [768, 4096] = sum_j exp(q k_j^T / sqrt(H)) v_j   (fp32)
  den_partial  [4096]      = sum_j exp(q k_j^T / sqrt(H))
The host combines the two partials of each pair: (o0 + o1) / (d0 + d1).
No max-subtraction is needed: scores here are O(1), exp is safe in fp16/fp32,
and both partials use the same (absent) shift so the combine is exact.

On-chip layout ("transposed flash attention"):
  - qkv computed in head-major layout qT/kT [H, n] via lhsT=W, rhs=xT; v in
    [n, H] via lhsT=xT, rhs=W (no on-chip transposes anywhere).
  - scores computed transposed: sT[j, i] = (kT j-tile).T @ qT -> PSUM,
    exp via ScalarE (scale folded in), p stored fp16.
  - denominator: S = sum_jt p_jt accumulated on VectorE (fp32, then cast to
    fp16), then one ones[128,128] @ S matmul per i-block broadcasts
    den[i] across partitions in PSUM; row 0 is shipped to the host.
  - outT[h, i] += (v j-tile).T @ p accumulated over j in PSUM.  Output
    h-tiles are split in two groups of 3 so group g's PSUM banks are
    evacuated while group g+1's matmuls run (PE never stalls).
PSUM budget (8 banks): scores 1 + denominator 1 + out accumulators 6.
"""

import math
from contextlib import ExitStack
from functools import lru_cache

import numpy as np

import concourse.bacc as bacc
import concourse.bass as bass
import concourse.tile as tile
from concourse import mybir
from concourse.bass_utils import run_bass_kernel_spmd

B, N, C = 4, 4096, 768
H = 768          # head dim (== C)
H3 = 3 * H
NCORES = 8
NK = N // 2      # keys per core
DT = mybir.dt.float16
F32 = mybir.dt.float32
SCALE = 1.0 / math.sqrt(H)

CT = C // 128    # 6 contraction tiles (c)
HT = H // 128    # 6 head tiles (h)
HG = HT // 2     # 3 h-tiles per output group
JT = NK // 128   # 16 key tiles (j) per core
RB = 8           # r-blocks of 512 over the 4096 rows
RBS = N // RB    # 512
KRB = RB // 2    # r-blocks that contain this core's keys (first 4)
IB = 8           # i-blocks of 512 over all 4096 queries
IBS = N // IB    # 512


def build_program():
    nc = bacc.Bacc(
        "TRN2",
        target_bir_lowering=False,
        debug=False,
        enable_asserts=False,
        num_devices=NCORES,
    )
    xT_d = nc.dram_tensor("xT", [C, N], DT, kind="ExternalInput").ap()
    w_d = nc.dram_tensor("w", [C, H3], DT, kind="ExternalInput").ap()
    bqk_d = nc.dram_tensor("bqk", [128, 2 * HT], F32, kind="ExternalInput").ap()
    bv_d = nc.dram_tensor("bv", [128, H], F32, kind="ExternalInput").ap()
    outT_d = nc.dram_tensor("outT", [H, N], F32, kind="ExternalOutput").ap()
    den_d = nc.dram_tensor("den", [IB, IBS], F32, kind="ExternalOutput").ap()

    with tile.TileContext(nc) as tc:
        with ExitStack() as ctx:
            persist = ctx.enter_context(tc.tile_pool(name="persist", bufs=1))

            kT = [persist.tile([128, NK], DT, tag=f"kT{t}", name=f"kT{t}")
                  for t in range(HT)]
            qT = [persist.tile([128, N], DT, tag=f"qT{t}", name=f"qT{t}")
                  for t in range(HT)]
            vv = [persist.tile([128, H], DT, tag=f"v{t}", name=f"v{t}")
                  for t in range(JT)]
            ones = persist.tile([128, 128], DT, tag="ones")
            nc.vector.memset(ones, 1.0)
            bqk = persist.tile([128, 2 * HT], F32, tag="bqk")
            nc.sync.dma_start(out=bqk, in_=bqk_d)
            bvb = persist.tile([128, H], F32, tag="bvb")
            nc.sync.dma_start(out=bvb, in_=bv_d)

            # ---- Phase 1: QKV projection ----
            with tc.tile_pool(name="wpool", bufs=1) as wpool, \
                 tc.tile_pool(name="xpool", bufs=2 * CT) as xpool, \
                 tc.tile_pool(name="pj", bufs=4, space="PSUM") as pj, \
                 tc.tile_pool(name="pv", bufs=2, space="PSUM") as pv:

                ws = [wpool.tile([128, H3], DT, tag=f"w{t}", name=f"w{t}")
                      for t in range(CT)]

                def load_xt(rb):
                    r0 = rb * RBS
                    tiles = []
                    for ct in range(CT):
                        t = xpool.tile([128, RBS], DT, tag="xt", name=f"xt{rb}_{ct}")
                        nc.sync.dma_start(
                            out=t, in_=xT_d[ct * 128:(ct + 1) * 128, r0:r0 + RBS])
                        tiles.append(t)
                    return tiles

                # first r-block's x and the k-projection columns of W land
                # first so the PE can start as early as possible
                xt0 = load_xt(0)
                for ht in range(HT):
                    for ct in range(CT):
                        nc.sync.dma_start(
                            out=ws[ct][:, H + ht * 128: H + (ht + 1) * 128],
                            in_=w_d[ct * 128:(ct + 1) * 128,
                                    H + ht * 128: H + (ht + 1) * 128])
                for ct in range(CT):
                    nc.sync.dma_start(out=ws[ct][:, 0:H],
                                      in_=w_d[ct * 128:(ct + 1) * 128, 0:H])
                    nc.sync.dma_start(out=ws[ct][:, 2 * H:H3],
                                      in_=w_d[ct * 128:(ct + 1) * 128, 2 * H:H3])

                for rb in range(RB):
                    r0 = rb * RBS
                    xt = xt0 if rb == 0 else load_xt(rb)

                    projs = [(0, qT, r0)]          # q: every r-block
                    if rb < KRB:
                        projs.insert(0, (H, kT, r0))   # k: first half only
                    for (wofs, dst, c0) in projs:
                        for ht in range(HT):
                            ps = pj.tile([128, RBS], F32, tag="pj")
                            for ct in range(CT):
                                nc.tensor.matmul(
                                    ps,
                                    ws[ct][:, wofs + ht * 128: wofs + (ht + 1) * 128],
                                    xt[ct],
                                    start=(ct == 0), stop=(ct == CT - 1),
                                )
                            bcol = (0 if wofs == 0 else HT) + ht
                            nc.scalar.activation(
                                out=dst[ht][:, c0:c0 + RBS],
                                in_=ps,
                                func=mybir.ActivationFunctionType.Identity,
                                bias=bqk[:, bcol:bcol + 1],
                            )

                    if rb < KRB:
                        for j in range(RBS // 128):
                            jt = rb * (RBS // 128) + j
                            ps = pv.tile([128, H], F32, tag="pv")
                            for ct in range(CT):
                                xs = xt[ct][:, j * 128:(j + 1) * 128]
                                nc.tensor.matmul(
                                    ps[:, 0:512], xs, ws[ct][:, 2 * H: 2 * H + 512],
                                    start=(ct == 0), stop=(ct == CT - 1))
                                nc.tensor.matmul(
                                    ps[:, 512:H], xs, ws[ct][:, 2 * H + 512: 3 * H],
                                    start=(ct == 0), stop=(ct == CT - 1))
                            nc.vector.tensor_add(vv[jt], ps, bvb)

            # ---- Phase 2: attention (partial sums over this core's keys) ----
            # Software-pipelined: the PV matmuls for j-tile jt are emitted
            # after the QK matmuls for jt+1, so the PE always has ~1.3us of
            # work covering the exp (ScalarE) of the score tile it just
            # produced (score pool is single-buffered: 1+1+6 = 8 PSUM banks).
            # Denominator matmul and PSUM evacuations of i-block N are
            # emitted between the first QK/PV groups of i-block N+1, where
            # their inputs are certainly ready; evacuation copies are split
            # between VectorE and ScalarE so they hide under the PE work.
            with tc.tile_pool(name="ppool", bufs=1) as ppool, \
                 tc.tile_pool(name="opool", bufs=8) as opool, \
                 tc.tile_pool(name="spool", bufs=2) as spool, \
                 tc.tile_pool(name="ps_s", bufs=1, space="PSUM") as ps_s, \
                 tc.tile_pool(name="ps_d", bufs=1, space="PSUM") as ps_d, \
                 tc.tile_pool(name="ps_o", bufs=6, space="PSUM") as ps_o:
                p_t = [ppool.tile([128, IBS], DT, tag=f"p{t}", name=f"p{t}")
                       for t in range(JT)]

                pending = []   # deferred work, flushed between PE groups

                def flush():
                    while pending:
                        pending.pop(0)()

                def emit_pv(og, jt, i0):
                    def go():
                        for ht in range(HT):
                            nc.tensor.matmul(
                                og[ht],
                                vv[jt][:, ht * 128:(ht + 1) * 128],
                                p_t[jt],
                                start=(jt == 0), stop=(jt == JT - 1),
                            )
                    pending.append(go)

                def emit_den_and_evac(og, S16, ib, i0):
                    def go():
                        dps = ps_d.tile([128, IBS], F32, tag="d")
                        nc.tensor.matmul(dps, ones, S16, start=True, stop=True)
                        dt_sb = opool.tile([1, IBS], F32, tag="dt",
                                           name=f"dt{ib}")
                        nc.vector.tensor_copy(out=dt_sb, in_=dps[0:1, :])
                        nc.sync.dma_start(out=den_d[ib:ib + 1, :], in_=dt_sb)
                        for ht in range(HT):
                            ot = opool.tile([128, IBS], F32, tag="ot",
                                            name=f"ot{i0}_{ht}")
                            if ht % 2 == 0:
                                nc.vector.tensor_copy(out=ot, in_=og[ht])
                            else:
                                nc.scalar.activation(
                                    out=ot, in_=og[ht],
                                    func=mybir.ActivationFunctionType.Copy)
                            nc.sync.dma_start(
                                out=outT_d[ht * 128:(ht + 1) * 128,
                                           i0:i0 + IBS],
                                in_=ot)
                    pending.append(go)

                for ib in range(IB):
                    i0 = ib * IBS
                    og = [ps_o.tile([128, IBS], F32, tag="o", name=f"o{ib}_{g}")
                          for g in range(HT)]
                    Sf = spool.tile([128, IBS], F32, tag="Sf", name=f"Sf{ib}")
                    for jt in range(JT):
                        sps = ps_s.tile([128, IBS], F32, tag="s")
                        for ht in range(HT):
                            nc.tensor.matmul(
                                sps,
                                kT[ht][:, jt * 128:(jt + 1) * 128],
                                qT[ht][:, i0:i0 + IBS],
                                start=(ht == 0), stop=(ht == HT - 1),
                            )
                        flush()
                        nc.scalar.activation(
                            out=p_t[jt], in_=sps,
                            func=mybir.ActivationFunctionType.Exp,
                            scale=SCALE,
                        )
                        if jt == 0:
                            nc.vector.tensor_copy(out=Sf, in_=p_t[jt])
                        else:
                            nc.vector.tensor_add(Sf, Sf, p_t[jt])
                        emit_pv(og, jt, i0)
                    S16 = spool.tile([128, IBS], DT, tag="S16", name=f"S16{ib}")
                    nc.vector.tensor_copy(out=S16, in_=Sf)
                    emit_den_and_evac(og, S16, ib, i0)
                flush()
    nc.compile()
    return nc


@lru_cache(maxsize=1)
def _cached_program():
    return build_program()


def _prep_in_maps(x, W_qkv, b_qkv):
    w16 = W_qkv.astype(np.float16)
    bq = b_qkv[0:H].astype(np.float32).reshape(HT, 128).T    # [128, HT]
    bk = b_qkv[H:2 * H].astype(np.float32).reshape(HT, 128).T
    bqk = np.ascontiguousarray(np.concatenate([bq, bk], axis=1))  # [128, 2*HT]
    bv = np.ascontiguousarray(
        np.broadcast_to(b_qkv[2 * H:3 * H].astype(np.float32), (128, H)))

    in_maps = []
    for core in range(NCORES):
        b, kh = core // 2, core % 2
        xb = x[b]  # [N, C] f32
        if kh == 1:
            # Rotate so this core's key rows occupy rows [0, NK). Queries are
            # also rotated; the host rotates this core's outputs back.
            xb = np.concatenate([xb[NK:], xb[:NK]], axis=0)
        xT = np.ascontiguousarray(xb.T).astype(np.float16)
        in_maps.append({"xT": xT, "w": w16, "bqk": bqk, "bv": bv})
    return in_maps


def _combine(results):
    out = np.empty((B, N, C), dtype=np.float32)
    for b in range(B):
        o0 = results[2 * b]["outT"]              # [H, N]
        d0 = results[2 * b]["den"].reshape(N)    # [N]
        o1 = results[2 * b + 1]["outT"]
        d1 = results[2 * b + 1]["den"].reshape(N)
        # core (2b+1) worked in rotated query order; rotate back
        o1 = np.concatenate([o1[:, NK:], o1[:, :NK]], axis=1)
        d1 = np.concatenate([d1[NK:], d1[:NK]])
        out[b] = ((o0 + o1) / (d0 + d1)).T
    return out


def kernel(x, W_qkv, b_qkv):
    nc = _cached_program()
    in_maps = _prep_in_maps(x, W_qkv, b_qkv)
    res = run_bass_kernel_spmd(nc, in_maps, core_ids=list(range(NCORES)))
    return _combine(res.results)


# revision 13
# speedup vs baseline: 1.2066x; 1.0024x over previous
"""Fused single-head attention (QKV projection + softmax(QK^T)V) on 8 trn2 cores.

Problem (hardcoded): x [4, 4096, 768] f32, W_qkv [768, 2304] f32, b_qkv [2304] f32.
  qkv = x @ W_qkv + b_qkv ; q,k,v = split(qkv, 3)
  out = softmax(q k^T / sqrt(768)) v          -> [4, 4096, 768] f32

Sharding: batch (4) x key-halves (2) -> 8 cores. Each core gets one batch's
x (pre-transposed on host to xT [768, 4096] fp16, with the key half it owns
rotated into columns [0, 2048)), projects q for all 4096 queries but k/v only
for its 2048 keys, and computes PARTIAL attention sums over those keys:
  outT_partial [768, 4096] = sum_j exp(q k_j^T / sqrt(H)) v_j   (fp32)
  den_partial  [4096]      = sum_j exp(q k_j^T / sqrt(H))
The host combines the two partials of each pair: (o0 + o1) / (d0 + d1).
No max-subtraction is needed: scores here are O(1), exp is safe in fp16/fp32,
and both partials use the same (absent) shift so the combine is exact.

On-chip layout ("transposed flash attention"):
  - qkv computed in head-major layout qT/kT [H, n] via lhsT=W, rhs=xT; v in
    [n, H] via lhsT=xT, rhs=W (no on-chip transposes anywhere).
  - scores computed transposed: sT[j, i] = (kT j-tile).T @ qT -> PSUM,
    exp via ScalarE (scale folded in), p stored fp16.
  - denominator: S = sum_jt p_jt accumulated on VectorE (fp32, then cast to
    fp16), then one ones[128,128] @ S matmul per i-block broadcasts
    den[i] across partitions in PSUM; row 0 is shipped to the host.
  - outT[h, i] += (v j-tile).T @ p accumulated over j in PSUM.  Output
    h-tiles are split in two groups of 3 so group g's PSUM banks are
    evacuated while group g+1's matmuls run (PE never stalls).
PSUM budget (8 banks): scores 1 + denominator 1 + out accumulators 6.
"""

import math
from contextlib import ExitStack
from functools import lru_cache

import numpy as np

import concourse.bacc as bacc
import concourse.bass as bass
import concourse.tile as tile
from concourse import mybir
from concourse.bass_utils import run_bass_kernel_spmd

B, N, C = 4, 4096, 768
H = 768          # head dim (== C)
H3 = 3 * H
NCORES = 8
NK = N // 2      # keys per core
DT = mybir.dt.float16
F32 = mybir.dt.float32
SCALE = 1.0 / math.sqrt(H)

CT = C // 128    # 6 contraction tiles (c)
HT = H // 128    # 6 head tiles (h)
HG = HT // 2     # 3 h-tiles per output group
JT = NK // 128   # 16 key tiles (j) per core
RB = 8           # r-blocks of 512 over the 4096 rows
RBS = N // RB    # 512
KRB = RB // 2    # r-blocks that contain this core's keys (first 4)
IB = 8           # i-blocks of 512 over all 4096 queries
IBS = N // IB    # 512


def build_program():
    nc = bacc.Bacc(
        "TRN2",
        target_bir_lowering=False,
        debug=False,
        enable_asserts=False,
        num_devices=NCORES,
    )
    xT_d = nc.dram_tensor("xT", [C, N], DT, kind="ExternalInput").ap()
    w_d = nc.dram_tensor("w", [C, H3], DT, kind="ExternalInput").ap()
    bqk_d = nc.dram_tensor("bqk", [128, 2 * HT], F32, kind="ExternalInput").ap()
    bv_d = nc.dram_tensor("bv", [128, H], F32, kind="ExternalInput").ap()
    outT_d = nc.dram_tensor("outT", [H, N], F32, kind="ExternalOutput").ap()
    den_d = nc.dram_tensor("den", [IB, IBS], F32, kind="ExternalOutput").ap()

    with tile.TileContext(nc) as tc:
        with ExitStack() as ctx:
            persist = ctx.enter_context(tc.tile_pool(name="persist", bufs=1))

            kT = [persist.tile([128, NK], DT, tag=f"kT{t}", name=f"kT{t}")
                  for t in range(HT)]
            qT = [persist.tile([128, N], DT, tag=f"qT{t}", name=f"qT{t}")
                  for t in range(HT)]
            vv = [persist.tile([128, H], DT, tag=f"v{t}", name=f"v{t}")
                  for t in range(JT)]
            ones = persist.tile([128, 128], DT, tag="ones")
            nc.vector.memset(ones, 1.0)
            bqk = persist.tile([128, 2 * HT], F32, tag="bqk")
            bvb = persist.tile([128, H], F32, tag="bvb")

            # ---- Phase 1: QKV projection ----
            with tc.tile_pool(name="wpool", bufs=1) as wpool, \
                 tc.tile_pool(name="xpool", bufs=3 * CT) as xpool, \
                 tc.tile_pool(name="pj", bufs=4, space="PSUM") as pj, \
                 tc.tile_pool(name="pv", bufs=2, space="PSUM") as pv:

                ws = [wpool.tile([128, H3], DT, tag=f"w{t}", name=f"w{t}")
                      for t in range(CT)]

                def load_xt(rb):
                    r0 = rb * RBS
                    tiles = []
                    for ct in range(CT):
                        t = xpool.tile([128, RBS], DT, tag="xt", name=f"xt{rb}_{ct}")
                        nc.sync.dma_start(
                            out=t, in_=xT_d[ct * 128:(ct + 1) * 128, r0:r0 + RBS])
                        tiles.append(t)
                    return tiles

                # DMA issue order = need order: first r-block's x, then the
                # k-projection h-tile-0 columns of W, then the small biases,
                # then the rest of W.
                xts = [None] * RB
                xts[0] = load_xt(0)
                for ht in range(HT):
                    for ct in range(CT):
                        nc.sync.dma_start(
                            out=ws[ct][:, H + ht * 128: H + (ht + 1) * 128],
                            in_=w_d[ct * 128:(ct + 1) * 128,
                                    H + ht * 128: H + (ht + 1) * 128])
                    if ht == 0:
                        nc.sync.dma_start(out=bqk, in_=bqk_d)
                for ct in range(CT):
                    nc.sync.dma_start(out=ws[ct][:, 0:H],
                                      in_=w_d[ct * 128:(ct + 1) * 128, 0:H])
                    nc.sync.dma_start(out=ws[ct][:, 2 * H:H3],
                                      in_=w_d[ct * 128:(ct + 1) * 128, 2 * H:H3])
                nc.sync.dma_start(out=bvb, in_=bv_d)

                for rb in range(RB):
                    r0 = rb * RBS
                    if rb + 1 < RB:
                        xts[rb + 1] = load_xt(rb + 1)
                    xt = xts[rb]

                    projs = [(0, qT, r0)]          # q: every r-block
                    if rb < KRB:
                        projs.insert(0, (H, kT, r0))   # k: first half only
                    for (wofs, dst, c0) in projs:
                        for ht in range(HT):
                            ps = pj.tile([128, RBS], F32, tag="pj")
                            for ct in range(CT):
                                nc.tensor.matmul(
                                    ps,
                                    ws[ct][:, wofs + ht * 128: wofs + (ht + 1) * 128],
                                    xt[ct],
                                    start=(ct == 0), stop=(ct == CT - 1),
                                )
                            bcol = (0 if wofs == 0 else HT) + ht
                            nc.scalar.activation(
                                out=dst[ht][:, c0:c0 + RBS],
                                in_=ps,
                                func=mybir.ActivationFunctionType.Identity,
                                bias=bqk[:, bcol:bcol + 1],
                            )

                    if rb < KRB:
                        for j in range(RBS // 128):
                            jt = rb * (RBS // 128) + j
                            ps = pv.tile([128, H], F32, tag="pv")
                            for ct in range(CT):
                                xs = xt[ct][:, j * 128:(j + 1) * 128]
                                nc.tensor.matmul(
                                    ps[:, 0:512], xs, ws[ct][:, 2 * H: 2 * H + 512],
                                    start=(ct == 0), stop=(ct == CT - 1))
                                nc.tensor.matmul(
                                    ps[:, 512:H], xs, ws[ct][:, 2 * H + 512: 3 * H],
                                    start=(ct == 0), stop=(ct == CT - 1))
                            nc.vector.tensor_add(vv[jt], ps, bvb)

            # ---- Phase 2: attention (partial sums over this core's keys) ----
            # Software-pipelined: the PV matmuls for j-tile jt are emitted
            # after the QK matmuls for jt+1, so the PE always has ~1.3us of
            # work covering the exp (ScalarE) of the score tile it just
            # produced (score pool is single-buffered: 1+1+6 = 8 PSUM banks).
            # Denominator matmul and PSUM evacuations of i-block N are
            # emitted between the first QK/PV groups of i-block N+1, where
            # their inputs are certainly ready; evacuation copies are split
            # between VectorE and ScalarE so they hide under the PE work.
            with tc.tile_pool(name="ppool", bufs=1) as ppool, \
                 tc.tile_pool(name="opool", bufs=8) as opool, \
                 tc.tile_pool(name="spool", bufs=2) as spool, \
                 tc.tile_pool(name="ps_s", bufs=1, space="PSUM") as ps_s, \
                 tc.tile_pool(name="ps_d", bufs=1, space="PSUM") as ps_d, \
                 tc.tile_pool(name="ps_o", bufs=6, space="PSUM") as ps_o:
                p_t = [ppool.tile([128, IBS], DT, tag=f"p{t}", name=f"p{t}")
                       for t in range(JT)]

                pending = []   # deferred work, flushed between PE groups

                def flush():
                    while pending:
                        pending.pop(0)()

                def emit_pv(og, jt, i0):
                    def go():
                        for ht in range(HT):
                            nc.tensor.matmul(
                                og[ht],
                                vv[jt][:, ht * 128:(ht + 1) * 128],
                                p_t[jt],
                                start=(jt == 0), stop=(jt == JT - 1),
                            )
                    pending.append(go)

                def emit_den_and_evac(og, S16, ib, i0):
                    def go():
                        dps = ps_d.tile([128, IBS], F32, tag="d")
                        nc.tensor.matmul(dps, ones, S16, start=True, stop=True)
                        dt_sb = opool.tile([1, IBS], F32, tag="dt",
                                           name=f"dt{ib}")
                        nc.vector.tensor_copy(out=dt_sb, in_=dps[0:1, :])
                        nc.sync.dma_start(out=den_d[ib:ib + 1, :], in_=dt_sb)
                        for ht in range(HT):
                            ot = opool.tile([128, IBS], F32, tag="ot",
                                            name=f"ot{i0}_{ht}")
                            if ht % 2 == 0:
                                nc.vector.tensor_copy(out=ot, in_=og[ht])
                            else:
                                nc.scalar.activation(
                                    out=ot, in_=og[ht],
                                    func=mybir.ActivationFunctionType.Copy)
                            nc.sync.dma_start(
                                out=outT_d[ht * 128:(ht + 1) * 128,
                                           i0:i0 + IBS],
                                in_=ot)
                    pending.append(go)

                for ib in range(IB):
                    i0 = ib * IBS
                    og = [ps_o.tile([128, IBS], F32, tag="o", name=f"o{ib}_{g}")
                          for g in range(HT)]
                    Sf = spool.tile([128, IBS], F32, tag="Sf", name=f"Sf{ib}")
                    for jt in range(JT):
                        sps = ps_s.tile([128, IBS], F32, tag="s")
                        for ht in range(HT):
                            nc.tensor.matmul(
                                sps,
                                kT[ht][:, jt * 128:(jt + 1) * 128],
                                qT[ht][:, i0:i0 + IBS],
                                start=(ht == 0), stop=(ht == HT - 1),
                            )
                        flush()
                        nc.scalar.activation(
                            out=p_t[jt], in_=sps,
                            func=mybir.ActivationFunctionType.Exp,
                            scale=SCALE,
                        )
                        if jt == 0:
                            nc.vector.tensor_copy(out=Sf, in_=p_t[jt])
                        else:
                            nc.vector.tensor_add(Sf, Sf, p_t[jt])
                        emit_pv(og, jt, i0)
                    S16 = spool.tile([128, IBS], DT, tag="S16", name=f"S16{ib}")
                    nc.vector.tensor_copy(out=S16, in_=Sf)
                    emit_den_and_evac(og, S16, ib, i0)
                flush()
    nc.compile()
    return nc


@lru_cache(maxsize=1)
def _cached_program():
    return build_program()


def _prep_in_maps(x, W_qkv, b_qkv):
    w16 = W_qkv.astype(np.float16)
    bq = b_qkv[0:H].astype(np.float32).reshape(HT, 128).T    # [128, HT]
    bk = b_qkv[H:2 * H].astype(np.float32).reshape(HT, 128).T
    bqk = np.ascontiguousarray(np.concatenate([bq, bk], axis=1))  # [128, 2*HT]
    bv = np.ascontiguousarray(
        np.broadcast_to(b_qkv[2 * H:3 * H].astype(np.float32), (128, H)))

    in_maps = []
    for core in range(NCORES):
        b, kh = core // 2, core % 2
        xb = x[b]  # [N, C] f32
        if kh == 1:
            # Rotate so this core's key rows occupy rows [0, NK). Queries are
            # also rotated; the host rotates this core's outputs back.
            xb = np.concatenate([xb[NK:], xb[:NK]], axis=0)
        xT = np.ascontiguousarray(xb.T).astype(np.float16)
        in_maps.append({"xT": xT, "w": w16, "bqk": bqk, "bv": bv})
    return in_maps


def _combine(results):
    out = np.empty((B, N, C), dtype=np.float32)
    for b in range(B):
        o0 = results[2 * b]["outT"]              # [H, N]
        d0 = results[2 * b]["den"].reshape(N)    # [N]
        o1 = results[2 * b + 1]["outT"]
        d1 = results[2 * b + 1]["den"].reshape(N)
        # core (2b+1) worked in rotated query order; rotate back
        o1 = np.concatenate([o1[:, NK:], o1[:, :NK]], axis=1)
        d1 = np.concatenate([d1[NK:], d1[:NK]])
        out[b] = ((o0 + o1) / (d0 + d1)).T
    return out


def kernel(x, W_qkv, b_qkv):
    nc = _cached_program()
    in_maps = _prep_in_maps(x, W_qkv, b_qkv)
    res = run_bass_kernel_spmd(nc, in_maps, core_ids=list(range(NCORES)))
    return _combine(res.results)


# revision 16
# speedup vs baseline: 1.2077x; 1.0009x over previous
"""Fused single-head attention (QKV projection + softmax(QK^T)V) on 8 trn2 cores.

Problem (hardcoded): x [4, 4096, 768] f32, W_qkv [768, 2304] f32, b_qkv [2304] f32.
  qkv = x @ W_qkv + b_qkv ; q,k,v = split(qkv, 3)
  out = softmax(q k^T / sqrt(768)) v          -> [4, 4096, 768] f32

Sharding: batch (4) x key-halves (2) -> 8 cores, no cross-core traffic.
Each core gets one batch's x (pre-transposed on host to xT [768, 4096] fp16,
with the key half it owns rotated into columns [0, 2048)), projects q for
all 4096 queries but k/v only for its 2048 keys, and computes PARTIAL
attention sums over those keys:
  outT_partial [768, 4096] = sum_j exp(q k_j^T / sqrt(H)) v_j   (fp32)
  den_partial  [4096]      = sum_j exp(q k_j^T / sqrt(H))
The host combines the two partials of each pair: (o0 + o1) / (d0 + d1).
No max-subtraction is needed: scores here are O(1), exp is safe in
fp16/fp32, and both partials use the same (absent) shift so the combine is
exact softmax.

On-chip layout ("transposed flash attention"):
  - qkv computed in head-major layout qT/kT [H, n] via lhsT=W, rhs=xT; v in
    [n, H] via lhsT=xT, rhs=W (no on-chip transposes anywhere).
  - scores computed transposed: sT[j, i] = (kT j-tile).T @ qT -> PSUM,
    exp via ScalarE (scale folded in), p stored fp16.
  - denominator: S = sum_jt p_jt accumulated on VectorE (fp32, then cast to
    fp16), then one ones[128,128] @ S matmul per i-block broadcasts
    den[i] across partitions in PSUM; row 0 is shipped to the host.
  - outT[h, i] += (v j-tile).T @ p accumulated over j in PSUM.
  - PV matmuls run one j-tile behind QK (software pipeline) so the PE
    always has work covering the exp; PSUM evacuations of i-block N are
    deferred into i-block N+1 and split across VectorE/ScalarE.
PSUM budget (8 banks): scores 1 + denominator 1 + out accumulators 6.
"""

import math
from contextlib import ExitStack
from functools import lru_cache

import numpy as np

import concourse.bacc as bacc
import concourse.bass as bass
import concourse.tile as tile
from concourse import mybir
from concourse.bass_utils import run_bass_kernel_spmd

B, N, C = 4, 4096, 768
H = 768          # head dim (== C)
H3 = 3 * H
NCORES = 8
NK = N // 2      # keys per core
DT = mybir.dt.float16
F32 = mybir.dt.float32
SCALE = 1.0 / math.sqrt(H)

CT = C // 128    # 6 contraction tiles (c)
HT = H // 128    # 6 head tiles (h)
JT = NK // 128   # 16 key tiles (j) per core
RB = 8           # r-blocks of 512 over the 4096 rows
RBS = N // RB    # 512
KRB = RB // 2    # r-blocks that contain this core's keys (first 4)
IB = 8           # i-blocks of 512 over all 4096 queries
IBS = N // IB    # 512


def build_program():
    nc = bacc.Bacc(
        "TRN2",
        target_bir_lowering=False,
        debug=False,
        enable_asserts=False,
        num_devices=NCORES,
    )
    xT_d = nc.dram_tensor("xT", [C, N], DT, kind="ExternalInput").ap()
    w_d = nc.dram_tensor("w", [C, H3], DT, kind="ExternalInput").ap()
    bqk_d = nc.dram_tensor("bqk", [128, 2 * HT], F32, kind="ExternalInput").ap()
    bv_d = nc.dram_tensor("bv", [128, H], F32, kind="ExternalInput").ap()
    outT_d = nc.dram_tensor("outT", [H, N], F32, kind="ExternalOutput").ap()
    den_d = nc.dram_tensor("den", [IB, IBS], F32, kind="ExternalOutput").ap()

    with tile.TileContext(nc) as tc:
        with ExitStack() as ctx:
            persist = ctx.enter_context(tc.tile_pool(name="persist", bufs=1))

            kT = [persist.tile([128, NK], DT, tag=f"kT{t}", name=f"kT{t}")
                  for t in range(HT)]
            qT = [persist.tile([128, N], DT, tag=f"qT{t}", name=f"qT{t}")
                  for t in range(HT)]
            vv = [persist.tile([128, H], DT, tag=f"v{t}", name=f"v{t}")
                  for t in range(JT)]
            ones = persist.tile([128, 128], DT, tag="ones")
            nc.vector.memset(ones, 1.0)
            bqk = persist.tile([128, 2 * HT], F32, tag="bqk")
            bvb = persist.tile([128, H], F32, tag="bvb")

            # ---- Phase 1: QKV projection ----
            with tc.tile_pool(name="wpool", bufs=1) as wpool, \
                 tc.tile_pool(name="xpool", bufs=3 * CT) as xpool, \
                 tc.tile_pool(name="pj", bufs=4, space="PSUM") as pj, \
                 tc.tile_pool(name="pv", bufs=2, space="PSUM") as pv:

                ws = [wpool.tile([128, H3], DT, tag=f"w{t}", name=f"w{t}")
                      for t in range(CT)]

                def load_xt(rb):
                    r0 = rb * RBS
                    tiles = []
                    for ct in range(CT):
                        t = xpool.tile([128, RBS], DT, tag="xt", name=f"xt{rb}_{ct}")
                        nc.sync.dma_start(
                            out=t, in_=xT_d[ct * 128:(ct + 1) * 128, r0:r0 + RBS])
                        tiles.append(t)
                    return tiles

                # DMA issue order = need order: first r-block's x, then the
                # k-projection h-tile-0 columns of W, then the small biases,
                # then the rest of W.
                xts = [None] * RB
                xts[0] = load_xt(0)
                for ht in range(HT):
                    for ct in range(CT):
                        nc.sync.dma_start(
                            out=ws[ct][:, H + ht * 128: H + (ht + 1) * 128],
                            in_=w_d[ct * 128:(ct + 1) * 128,
                                    H + ht * 128: H + (ht + 1) * 128])
                    if ht == 0:
                        nc.sync.dma_start(out=bqk, in_=bqk_d)
                for ct in range(CT):
                    nc.sync.dma_start(out=ws[ct][:, 0:H],
                                      in_=w_d[ct * 128:(ct + 1) * 128, 0:H])
                    nc.sync.dma_start(out=ws[ct][:, 2 * H:H3],
                                      in_=w_d[ct * 128:(ct + 1) * 128, 2 * H:H3])
                nc.sync.dma_start(out=bvb, in_=bv_d)

                for rb in range(RB):
                    r0 = rb * RBS
                    if rb + 1 < RB:
                        xts[rb + 1] = load_xt(rb + 1)
                    xt = xts[rb]

                    projs = [(0, qT, r0)]          # q: every r-block
                    if rb < KRB:
                        projs.insert(0, (H, kT, r0))   # k: first half only
                    for (wofs, dst, c0) in projs:
                        for ht in range(HT):
                            ps = pj.tile([128, RBS], F32, tag="pj")
                            for ct in range(CT):
                                nc.tensor.matmul(
                                    ps,
                                    ws[ct][:, wofs + ht * 128: wofs + (ht + 1) * 128],
                                    xt[ct],
                                    start=(ct == 0), stop=(ct == CT - 1),
                                )
                            bcol = (0 if wofs == 0 else HT) + ht
                            nc.scalar.activation(
                                out=dst[ht][:, c0:c0 + RBS],
                                in_=ps,
                                func=mybir.ActivationFunctionType.Identity,
                                bias=bqk[:, bcol:bcol + 1],
                            )

                    if rb < KRB:
                        for j in range(RBS // 128):
                            jt = rb * (RBS // 128) + j
                            ps = pv.tile([128, H], F32, tag="pv")
                            for ct in range(CT):
                                xs = xt[ct][:, j * 128:(j + 1) * 128]
                                nc.tensor.matmul(
                                    ps[:, 0:512], xs, ws[ct][:, 2 * H: 2 * H + 512],
                                    start=(ct == 0), stop=(ct == CT - 1))
                                nc.tensor.matmul(
                                    ps[:, 512:H], xs, ws[ct][:, 2 * H + 512: 3 * H],
                                    start=(ct == 0), stop=(ct == CT - 1))
                            nc.vector.tensor_add(vv[jt], ps, bvb)

            # ---- Phase 2: attention (partial sums over this core's keys) ----
            with tc.tile_pool(name="ppool", bufs=1) as ppool, \
                 tc.tile_pool(name="opool", bufs=8) as opool, \
                 tc.tile_pool(name="spool", bufs=2) as spool, \
                 tc.tile_pool(name="ps_s", bufs=1, space="PSUM") as ps_s, \
                 tc.tile_pool(name="ps_d", bufs=1, space="PSUM") as ps_d, \
                 tc.tile_pool(name="ps_o", bufs=6, space="PSUM") as ps_o:
                p_t = [ppool.tile([128, IBS], DT, tag=f"p{t}", name=f"p{t}")
                       for t in range(JT)]

                pending = []   # deferred work, flushed between PE groups

                def flush():
                    while pending:
                        pending.pop(0)()

                def emit_pv(og, jt, i0):
                    def go():
                        for ht in range(HT):
                            nc.tensor.matmul(
                                og[ht],
                                vv[jt][:, ht * 128:(ht + 1) * 128],
                                p_t[jt],
                                start=(jt == 0), stop=(jt == JT - 1),
                            )
                    pending.append(go)

                def emit_den_and_evac(og, S16, ib, i0):
                    def go():
                        dps = ps_d.tile([128, IBS], F32, tag="d")
                        nc.tensor.matmul(dps, ones, S16, start=True, stop=True)
                        dt_sb = opool.tile([1, IBS], F32, tag="dt",
                                           name=f"dt{ib}")
                        nc.vector.tensor_copy(out=dt_sb, in_=dps[0:1, :])
                        nc.sync.dma_start(out=den_d[ib:ib + 1, :], in_=dt_sb)
                        for ht in range(HT):
                            ot = opool.tile([128, IBS], F32, tag="ot",
                                            name=f"ot{i0}_{ht}")
                            if ht % 2 == 0:
                                nc.vector.tensor_copy(out=ot, in_=og[ht])
                            else:
                                nc.scalar.activation(
                                    out=ot, in_=og[ht],
                                    func=mybir.ActivationFunctionType.Copy)
                            nc.sync.dma_start(
                                out=outT_d[ht * 128:(ht + 1) * 128,
                                           i0:i0 + IBS],
                                in_=ot)
                    pending.append(go)

                for ib in range(IB):
                    i0 = ib * IBS
                    og = [ps_o.tile([128, IBS], F32, tag="o", name=f"o{ib}_{g}")
                          for g in range(HT)]
                    Sf = spool.tile([128, IBS], F32, tag="Sf", name=f"Sf{ib}")
                    for jt in range(JT):
                        sps = ps_s.tile([128, IBS], F32, tag="s")
                        for ht in range(HT):
                            nc.tensor.matmul(
                                sps,
                                kT[ht][:, jt * 128:(jt + 1) * 128],
                                qT[ht][:, i0:i0 + IBS],
                                start=(ht == 0), stop=(ht == HT - 1),
                            )
                        flush()
                        nc.scalar.activation(
                            out=p_t[jt], in_=sps,
                            func=mybir.ActivationFunctionType.Exp,
                            scale=SCALE,
                        )
                        if jt == 0:
                            nc.vector.tensor_copy(out=Sf, in_=p_t[jt])
                        else:
                            nc.vector.tensor_add(Sf, Sf, p_t[jt])
                        emit_pv(og, jt, i0)
                    S16 = spool.tile([128, IBS], DT, tag="S16", name=f"S16{ib}")
                    nc.vector.tensor_copy(out=S16, in_=Sf)
                    emit_den_and_evac(og, S16, ib, i0)
                flush()
    nc.compile()
    return nc


@lru_cache(maxsize=1)
def _cached_program():
    return build_program()


def _prep_in_maps(x, W_qkv, b_qkv):
    w16 = W_qkv.astype(np.float16)
    bq = b_qkv[0:H].astype(np.float32).reshape(HT, 128).T    # [128, HT]
    bk = b_qkv[H:2 * H].astype(np.float32).reshape(HT, 128).T
    bqk = np.ascontiguousarray(np.concatenate([bq, bk], axis=1))  # [128, 2*HT]
    bv = np.ascontiguousarray(
        np.broadcast_to(b_qkv[2 * H:3 * H].astype(np.float32), (128, H)))

    in_maps = []
    for core in range(NCORES):
        b, kh = core // 2, core % 2
        xb = x[b]  # [N, C] f32
        if kh == 1:
            # Rotate so this core's key rows occupy rows [0, NK). Queries are
            # also rotated; the host rotates this core's outputs back.
            xb = np.concatenate([xb[NK:], xb[:NK]], axis=0)
        xT = np.ascontiguousarray(xb.T).astype(np.float16)
        in_maps.append({"xT": xT, "w": w16, "bqk": bqk, "bv": bv})
    return in_maps


def _combine(results):
    out = np.empty((B, N, C), dtype=np.float32)
    for b in range(B):
        o0 = results[2 * b]["outT"]              # [H, N]
        d0 = results[2 * b]["den"].reshape(N)    # [N]
        o1 = results[2 * b + 1]["outT"]
        d1 = results[2 * b + 1]["den"].reshape(N)
        # core (2b+1) worked in rotated query order; rotate back
        o1 = np.concatenate([o1[:, NK:], o1[:, :NK]], axis=1)
        d1 = np.concatenate([d1[NK:], d1[:NK]])
        out[b] = ((o0 + o1) / (d0 + d1)).T
    return out


def kernel(x, W_qkv, b_qkv):
    nc = _cached_program()
    in_maps = _prep_in_maps(x, W_qkv, b_qkv)
    res = run_bass_kernel_spmd(nc, in_maps, core_ids=list(range(NCORES)))
    return _combine(res.results)


# revision 22
# speedup vs baseline: 1.2166x; 1.0074x over previous
"""Fused single-head attention (QKV projection + softmax(QK^T)V) on 8 trn2 cores.

Problem (hardcoded): x [4, 4096, 768] f32, W_qkv [768, 2304] f32, b_qkv [2304] f32.
  qkv = x @ W_qkv + b_qkv ; q,k,v = split(qkv, 3)
  out = softmax(q k^T / sqrt(768)) v          -> [4, 4096, 768] f32

Sharding: batch (4) x key-halves (2) -> 8 cores, no cross-core traffic.
Each core gets one batch's x (pre-transposed on host to xT [768, 4096] fp16,
with the key half it owns rotated into columns [0, 2048)), projects q for
all 4096 queries but k/v only for its 2048 keys, and computes PARTIAL
attention sums over those keys:
  outT_partial [768, 4096] = sum_j exp(q k_j^T / sqrt(H)) v_j   (fp32)
  den_partial  [4096]      = sum_j exp(q k_j^T / sqrt(H))
The host combines the two partials of each pair: (o0 + o1) / (d0 + d1).
No max-subtraction is needed: scores here are O(1), exp is safe in
fp16/fp32, and both partials use the same (absent) shift so the combine is
exact softmax.

On-chip layout ("transposed flash attention"):
  - qkv computed in head-major layout qT/kT [H, n] via lhsT=W, rhs=xT; v in
    [n, H] via lhsT=xT, rhs=W (no on-chip transposes anywhere).
  - scores computed transposed: sT[j, i] = (kT j-tile).T @ qT -> PSUM,
    exp via ScalarE (scale folded in), p stored fp16.
  - denominator: S = sum_jt p_jt accumulated on VectorE (fp32, then cast to
    fp16), then one ones[128,128] @ S matmul per i-block broadcasts
    den[i] across partitions in PSUM; row 0 is shipped to the host.
  - outT[h, i] += (v j-tile).T @ p accumulated over j in PSUM.
  - PV matmuls run one j-tile behind QK (software pipeline) so the PE
    always has work covering the exp; PSUM evacuations of i-block N are
    deferred into i-block N+1 and split across VectorE/ScalarE.
PSUM budget (8 banks): scores 1 + denominator 1 + out accumulators 6.
"""

import math
from contextlib import ExitStack
from functools import lru_cache

import numpy as np

import concourse.bacc as bacc
import concourse.bass as bass
import concourse.tile as tile
from concourse import mybir
from concourse.bass_utils import run_bass_kernel_spmd

B, N, C = 4, 4096, 768
H = 768          # head dim (== C)
H3 = 3 * H
NCORES = 8
NK = N // 2      # keys per core
DT = mybir.dt.float16
F32 = mybir.dt.float32
SCALE = 1.0 / math.sqrt(H)

CT = C // 128    # 6 contraction tiles (c)
HT = H // 128    # 6 head tiles (h)
JT = NK // 128   # 16 key tiles (j) per core
RB = 8           # r-blocks of 512 over the 4096 rows
RBS = N // RB    # 512
KRB = RB // 2    # r-blocks that contain this core's keys (first 4)
IB = 8           # i-blocks of 512 over all 4096 queries
IBS = N // IB    # 512


def build_program():
    nc = bacc.Bacc(
        "TRN2",
        target_bir_lowering=False,
        debug=False,
        enable_asserts=False,
        num_devices=NCORES,
    )
    xT_d = nc.dram_tensor("xT", [C, N], DT, kind="ExternalInput").ap()
    w_d = nc.dram_tensor("w", [C, H3], DT, kind="ExternalInput").ap()
    bqk_d = nc.dram_tensor("bqk", [128, 2 * HT], F32, kind="ExternalInput").ap()
    bv_d = nc.dram_tensor("bv", [128, H], F32, kind="ExternalInput").ap()
    outT_d = nc.dram_tensor("outT", [H, N], F32, kind="ExternalOutput").ap()
    # per-partition partial softmax denominators; host sums over axis 1
    den_d = nc.dram_tensor("den", [IB, 128, IBS], DT, kind="ExternalOutput").ap()

    with tile.TileContext(nc) as tc:
        with ExitStack() as ctx:
            persist = ctx.enter_context(tc.tile_pool(name="persist", bufs=1))

            kT = [persist.tile([128, NK], DT, tag=f"kT{t}", name=f"kT{t}")
                  for t in range(HT)]
            qT = [persist.tile([128, N], DT, tag=f"qT{t}", name=f"qT{t}")
                  for t in range(HT)]
            vv = [persist.tile([128, H], DT, tag=f"v{t}", name=f"v{t}")
                  for t in range(JT)]
            bqk = persist.tile([128, 2 * HT], F32, tag="bqk")
            bvb = persist.tile([128, H], F32, tag="bvb")

            # ---- Phase 1: QKV projection ----
            with tc.tile_pool(name="wpool", bufs=1) as wpool, \
                 tc.tile_pool(name="xpool", bufs=3 * CT) as xpool, \
                 tc.tile_pool(name="pj", bufs=4, space="PSUM") as pj, \
                 tc.tile_pool(name="pv", bufs=2, space="PSUM") as pv:

                ws = [wpool.tile([128, H3], DT, tag=f"w{t}", name=f"w{t}")
                      for t in range(CT)]

                def load_xt(rb):
                    r0 = rb * RBS
                    tiles = []
                    for ct in range(CT):
                        t = xpool.tile([128, RBS], DT, tag="xt", name=f"xt{rb}_{ct}")
                        nc.sync.dma_start(
                            out=t, in_=xT_d[ct * 128:(ct + 1) * 128, r0:r0 + RBS])
                        tiles.append(t)
                    return tiles

                # DMA issue order = need order: first r-block's x, then the
                # k-projection h-tile-0 columns of W, then the small biases,
                # then the rest of W.
                # interleave so the ct=0 matmul's two inputs are the first
                # two DMAs in the queue, ct=1's the next two, ...
                xts = [None] * RB
                xt0 = []
                for ct in range(CT):
                    nc.sync.dma_start(
                        out=ws[ct][:, H: H + 128],
                        in_=w_d[ct * 128:(ct + 1) * 128, H: H + 128])
                    t = xpool.tile([128, RBS], DT, tag="xt", name=f"xt0_{ct}")
                    nc.sync.dma_start(out=t, in_=xT_d[ct * 128:(ct + 1) * 128, 0:RBS])
                    xt0.append(t)
                xts[0] = xt0
                for ht in range(1, HT):
                    for ct in range(CT):
                        nc.sync.dma_start(
                            out=ws[ct][:, H + ht * 128: H + (ht + 1) * 128],
                            in_=w_d[ct * 128:(ct + 1) * 128,
                                    H + ht * 128: H + (ht + 1) * 128])
                    if ht == 1:
                        nc.sync.dma_start(out=bqk, in_=bqk_d)
                for ct in range(CT):
                    nc.sync.dma_start(out=ws[ct][:, 0:H],
                                      in_=w_d[ct * 128:(ct + 1) * 128, 0:H])
                    nc.sync.dma_start(out=ws[ct][:, 2 * H:H3],
                                      in_=w_d[ct * 128:(ct + 1) * 128, 2 * H:H3])
                nc.sync.dma_start(out=bvb, in_=bv_d)

                for rb in range(RB):
                    r0 = rb * RBS
                    if rb + 1 < RB:
                        xts[rb + 1] = load_xt(rb + 1)
                    xt = xts[rb]

                    projs = [(0, qT, r0)]          # q: every r-block
                    if rb < KRB:
                        projs.insert(0, (H, kT, r0))   # k: first half only
                    for (wofs, dst, c0) in projs:
                        for ht in range(HT):
                            ps = pj.tile([128, RBS], F32, tag="pj")
                            for ct in range(CT):
                                nc.tensor.matmul(
                                    ps,
                                    ws[ct][:, wofs + ht * 128: wofs + (ht + 1) * 128],
                                    xt[ct],
                                    start=(ct == 0), stop=(ct == CT - 1),
                                )
                            bcol = (0 if wofs == 0 else HT) + ht
                            nc.scalar.activation(
                                out=dst[ht][:, c0:c0 + RBS],
                                in_=ps,
                                func=mybir.ActivationFunctionType.Identity,
                                bias=bqk[:, bcol:bcol + 1],
                            )

                    if rb < KRB:
                        for j in range(RBS // 128):
                            jt = rb * (RBS // 128) + j
                            ps = pv.tile([128, H], F32, tag="pv")
                            for ct in range(CT):
                                xs = xt[ct][:, j * 128:(j + 1) * 128]
                                nc.tensor.matmul(
                                    ps[:, 0:512], xs, ws[ct][:, 2 * H: 2 * H + 512],
                                    start=(ct == 0), stop=(ct == CT - 1))
                                nc.tensor.matmul(
                                    ps[:, 512:H], xs, ws[ct][:, 2 * H + 512: 3 * H],
                                    start=(ct == 0), stop=(ct == CT - 1))
                            nc.vector.tensor_add(vv[jt], ps, bvb)

            # ---- Phase 2: attention (partial sums over this core's keys) ----
            with tc.tile_pool(name="ppool", bufs=1) as ppool, \
                 tc.tile_pool(name="opool", bufs=8) as opool, \
                 tc.tile_pool(name="spool", bufs=2) as spool, \
                 tc.tile_pool(name="ps_s", bufs=2, space="PSUM") as ps_s, \
                 tc.tile_pool(name="ps_o", bufs=6, space="PSUM") as ps_o:
                p_t = [ppool.tile([128, IBS], DT, tag=f"p{t}", name=f"p{t}")
                       for t in range(JT)]

                pending = []   # deferred work, flushed between PE groups

                def flush():
                    while pending:
                        pending.pop(0)()

                def emit_pv(og, jt, i0):
                    def go():
                        for ht in range(HT):
                            nc.tensor.matmul(
                                og[ht],
                                vv[jt][:, ht * 128:(ht + 1) * 128],
                                p_t[jt],
                                start=(jt == 0), stop=(jt == JT - 1),
                            )
                    pending.append(go)

                def emit_den_and_evac(og, S16, ib, i0):
                    def go():
                        nc.sync.dma_start(out=den_d[ib], in_=S16)
                        for ht in range(HT):
                            ot = opool.tile([128, IBS], F32, tag="ot",
                                            name=f"ot{i0}_{ht}")
                            if ht % 2 == 0:
                                nc.vector.tensor_copy(out=ot, in_=og[ht])
                            else:
                                nc.scalar.activation(
                                    out=ot, in_=og[ht],
                                    func=mybir.ActivationFunctionType.Copy)
                            nc.sync.dma_start(
                                out=outT_d[ht * 128:(ht + 1) * 128,
                                           i0:i0 + IBS],
                                in_=ot)
                    pending.append(go)

                for ib in range(IB):
                    i0 = ib * IBS
                    og = [ps_o.tile([128, IBS], F32, tag="o", name=f"o{ib}_{g}")
                          for g in range(HT)]
                    Sf = spool.tile([128, IBS], F32, tag="Sf", name=f"Sf{ib}")
                    for jt in range(JT):
                        sps = ps_s.tile([128, IBS], F32, tag="s")
                        for ht in range(HT):
                            nc.tensor.matmul(
                                sps,
                                kT[ht][:, jt * 128:(jt + 1) * 128],
                                qT[ht][:, i0:i0 + IBS],
                                start=(ht == 0), stop=(ht == HT - 1),
                            )
                        flush()
                        nc.scalar.activation(
                            out=p_t[jt], in_=sps,
                            func=mybir.ActivationFunctionType.Exp,
                            scale=SCALE,
                        )
                        if jt == 0:
                            nc.vector.tensor_copy(out=Sf, in_=p_t[jt])
                        else:
                            nc.vector.tensor_add(Sf, Sf, p_t[jt])
                        emit_pv(og, jt, i0)
                    S16 = spool.tile([128, IBS], DT, tag="S16", name=f"S16{ib}")
                    nc.vector.tensor_copy(out=S16, in_=Sf)
                    emit_den_and_evac(og, S16, ib, i0)
                flush()
    nc.compile()
    return nc


@lru_cache(maxsize=1)
def _cached_program():
    return build_program()


def _prep_in_maps(x, W_qkv, b_qkv):
    w16 = W_qkv.astype(np.float16)
    bq = b_qkv[0:H].astype(np.float32).reshape(HT, 128).T    # [128, HT]
    bk = b_qkv[H:2 * H].astype(np.float32).reshape(HT, 128).T
    bqk = np.ascontiguousarray(np.concatenate([bq, bk], axis=1))  # [128, 2*HT]
    bv = np.ascontiguousarray(
        np.broadcast_to(b_qkv[2 * H:3 * H].astype(np.float32), (128, H)))

    in_maps = []
    for core in range(NCORES):
        b, kh = core // 2, core % 2
        xb = x[b]  # [N, C] f32
        if kh == 1:
            # Rotate so this core's key rows occupy rows [0, NK). Queries are
            # also rotated; the host rotates this core's outputs back.
            xb = np.concatenate([xb[NK:], xb[:NK]], axis=0)
        xT = np.ascontiguousarray(xb.T).astype(np.float16)
        in_maps.append({"xT": xT, "w": w16, "bqk": bqk, "bv": bv})
    return in_maps


def _combine(results):
    out = np.empty((B, N, C), dtype=np.float32)
    for b in range(B):
        o0 = results[2 * b]["outT"]              # [H, N]
        d0 = results[2 * b]["den"].astype(np.float32).sum(axis=1).reshape(N)
        o1 = results[2 * b + 1]["outT"]
        d1 = results[2 * b + 1]["den"].astype(np.float32).sum(axis=1).reshape(N)
        # core (2b+1) worked in rotated query order; rotate back
        o1 = np.concatenate([o1[:, NK:], o1[:, :NK]], axis=1)
        d1 = np.concatenate([d1[NK:], d1[:NK]])
        out[b] = ((o0 + o1) / (d0 + d1)).T
    return out


def kernel(x, W_qkv, b_qkv):
    nc = _cached_program()
    in_maps = _prep_in_maps(x, W_qkv, b_qkv)
    res = run_bass_kernel_spmd(nc, in_maps, core_ids=list(range(NCORES)))
    return _combine(res.results)
